# revision 1
# baseline (speedup 1.0000x reference)
"""Causal multi-head attention with RoPE on 8 TRN2 NeuronCores.

Problem (hardcoded): x [2, 2048, 1024] f32, W_qkv [1024, 3072], W_o [1024, 1024],
16 heads x 64 dh, RoPE base 10000, causal softmax attention, o-projection.

Sharding: core c = 4*b + g handles batch b (2) and head group g (4 heads).
Per core (all matmul data bf16, PSUM f32):
  - proj phase interleaved per 512-col seq block n: v tiles (natural layout,
    ones-columns trick for the softmax denominator), qkT pair-tiles
    (transposed [dh, seq], two head-pairs per 2-bank psum, one ACT copy),
    RoPE (rotate_half as +-1 block matmul on PE; psum consumed directly by
    DVE: t1=rot*sin, t2=q*cos, add -- no ACT copy)
  - attention chunk-major: for chunk i (512 q cols), 4 heads: full score
    tiles run in [128,1024] psum pairs with ONE exp per pair; the 4 ragged
    diagonal tiles are packed into two psum pairs ([0:512]+[512:896] and
    [0:256]+[256:384]) so the whole diagonal costs 2 exps; triangular mask
    multiply on DVE on the 128-wide ragged blocks only; P@V accumulates into
    a [128,512] psz whose rows 64-127 are the softmax denominator (v ones
    columns), so normalization is reciprocal + multiply
  - after each chunk: AllToAll round within the 4-core batch group
    (128 q cols/core) ships all 16 heads' z^T for that chunk; the local
    o-proj with full W_o is deferred into the middle of the next chunk so
    the collective latency hides under attention. Output dram is bf16.
Host reassembles: core (b,g), round i -> rows [512i+128g : +128] of batch b,
upcast to f32.
"""

import numpy as np
import ml_dtypes

import concourse.bass as bass
import concourse.mybir as mybir
import concourse.tile as tile
from concourse.bass_utils import run_bass_kernel_spmd

BF16 = mybir.dt.bfloat16
F32 = mybir.dt.float32
AF = mybir.ActivationFunctionType

B, S, D = 2, 2048, 1024
H, DH = 16, 64
HPC = 4            # heads per core
N_CORES = 8
ROPE_BASE = 10000.0

NT = S // 128      # 16 t-tiles of 128
RW = 64            # q cols per core per batch per a2a round (8-core a2a)


def _npbf(a):
    return np.ascontiguousarray(a).astype(ml_dtypes.bfloat16)


def split_excess_waits(nc, limit=1):
    """walrus codegen encodes at most ~1 sync wait on most instruction structs
    (Matmult-with-embedded-ldweights, CollectiveCompute, Drain...).  Move
    excess waits onto standalone EventSemaphore instructions just before, on
    the same engine; sequencers process instructions in order so semantics
    are identical."""
    for fn in nc.m.functions:
        for bb in fn.blocks:
            out = []
            for inst in bb.instructions:
                si = inst.sync_info
                waits = list(si.on_wait) if si is not None and si.on_wait else []
                if len(waits) > limit:
                    keep = waits[len(waits) - limit:]
                    for k, w in enumerate(waits[: len(waits) - limit]):
                        ev = mybir.InstEventSemaphore(name=f"{inst.name}-wsp{k}")
                        ev.engine = inst.engine
                        ev.sync_info = mybir.SyncInfo(on_wait=[w], on_update=[])
                        out.append(ev)
                    si.on_wait = keep
                    inst.sync_info = si
                out.append(inst)
            bb.instructions = out
    return nc


def build_nc(reps=1, for_sim=False, pair_bufs=3, pt_bufs=6, psz_bufs=2,
             oproj_after_head=3, use_divide=False, mask_pe=True,
             qkcopy_act=True, spool_bufs=4):
    nc = bass.Bass()

    xt = nc.declare_dram_parameter("xt", [D, S], BF16, isOutput=False)
    wqk = nc.declare_dram_parameter("wqk", [D, 512], BF16, isOutput=False)
    wv = nc.declare_dram_parameter("wv", [D, HPC * 64], BF16, isOutput=False)
    wo = nc.declare_dram_parameter("wo", [D, D], BF16, isOutput=False)
    cosp = nc.declare_dram_parameter("cosp", [128, S], BF16, isOutput=False)
    sinp = nc.declare_dram_parameter("sinp", [128, S], BF16, isOutput=False)
    rotm = nc.declare_dram_parameter("rotm", [128, 128], BF16, isOutput=False)
    trim = nc.declare_dram_parameter("trim", [128, 128], BF16, isOutput=False)
    idm = nc.declare_dram_parameter("idm", [128, 128], BF16, isOutput=False)
    out = nc.declare_dram_parameter("out", [4 * 2 * RW, D], BF16, isOutput=True)

    with tile.TileContext(nc) as tc:
        with (
            tc.tile_pool(name="const", bufs=1) as cpool,
            tc.tile_pool(name="work", bufs=1) as wpool,
            tc.tile_pool(name="str", bufs=spool_bufs) as spool,
            tc.tile_pool(name="ptp", bufs=pt_bufs) as ptpool,
            tc.tile_pool(name="ztp", bufs=2) as ztpool,
            tc.tile_pool(name="psA", bufs=pair_bufs, space="PSUM") as ppa,
            tc.tile_pool(name="psZ", bufs=psz_bufs, space="PSUM") as ppz,
            tc.tile_pool(name="dram", bufs=1, space="DRAM") as dpool,
        ):
            # ---- constant / input tiles ----
            wqk_sb = cpool.tile([128, 8, 512], BF16)
            wv_sb = cpool.tile([128, 8, HPC * 64], BF16)
            cos_sb = cpool.tile([128, S], BF16)
            sin_sb = cpool.tile([128, S], BF16)
            rot_sb = cpool.tile([128, 128], BF16)
            tri_sb = cpool.tile([128, 128], BF16)
            idm_sb = cpool.tile([128, 128], BF16)
            wo_sb = cpool.tile([128, 8, D], BF16)
            xt_sb = wpool.tile([128, 8, S], BF16)

            xt_r = xt.rearrange("(kd p) s -> p kd s", p=128)

            def loads():
                # ordered by first use (HWDGE issue and the DMA transfers are
                # effectively serial resources)
                nc.sync.dma_start(xt_sb[:, 0:4, 0:512], xt_r[:, 0:4, 0:512])
                nc.sync.dma_start(wv_sb[:], wv.rearrange("(kd p) e -> p kd e", p=128))
                nc.sync.dma_start(xt_sb[:, 4:8, 0:512], xt_r[:, 4:8, 0:512])
                nc.sync.dma_start(wqk_sb[:], wqk.rearrange("(kd p) e -> p kd e", p=128))
                nc.sync.dma_start(rot_sb[:], rotm[:])
                nc.sync.dma_start(cos_sb[:], cosp[:])
                nc.sync.dma_start(sin_sb[:], sinp[:])
                nc.sync.dma_start(xt_sb[:, :, 512:1024], xt_r[:, :, 512:1024])
                nc.sync.dma_start(tri_sb[:], trim[:])
                nc.sync.dma_start(idm_sb[:], idm[:])
                nc.sync.dma_start(xt_sb[:, :, 1024:1536], xt_r[:, :, 1024:1536])
                nc.sync.dma_start(xt_sb[:, :, 1536:2048], xt_r[:, :, 1536:2048])
                nc.sync.dma_start(wo_sb[:], wo.rearrange("(ft p) m -> p ft m", p=128))

            # ---- projection phase ----
            qk_raw = wpool.tile([128, 4, S], BF16)  # m=0..3: Qh01,Qh23,Kh01,Kh23
            qkr = wpool.tile([128, 4, S], BF16)
            v_sb = wpool.tile([128, NT, HPC * 128], BF16)
            v4 = v_sb[:].rearrange("p t (h e) -> p t h e", h=HPC)

            def v_ones_memset():
                # rows 64-127 of each per-head block = 1.0 -> P@V emits the
                # softmax denominator in psz rows 64-127
                nc.gpsimd.memset(v4[:, :, :, 64:128], 1.0)

            def v_tile(tt):
                ps = ppa.tile([128, 1024], F32, tag="pp")
                for kd in range(8):
                    nc.tensor.matmul(
                        ps[:, 0:HPC * 64],
                        xt_sb[:, kd, tt * 128:(tt + 1) * 128],
                        wv_sb[:, kd, :],
                        start=(kd == 0),
                        stop=(kd == 7),
                    )
                nc.vector.tensor_copy(
                    v4[:, tt, :, 0:64],
                    ps[:, 0:HPC * 64].rearrange("p (h e) -> p h e", h=HPC),
                )

            def qk_pair(n, a):
                # head-pairs m=2a, 2a+1 into one 2-bank psum, one ACT copy
                ps = ppa.tile([128, 1024], F32, tag="pp")
                sl = slice(n * 512, (n + 1) * 512)
                for half in range(2):
                    m = 2 * a + half
                    for kd in range(8):
                        nc.tensor.matmul(
                            ps[:, 512 * half:512 * half + 512],
                            wqk_sb[:, kd, m * 128:(m + 1) * 128],
                            xt_sb[:, kd, sl],
                            start=(kd == 0),
                            stop=(kd == 7),
                        )
                if qkcopy_act:
                    nc.scalar.activation(
                        qk_raw[:, 2 * a:2 * a + 2, sl],
                        ps[:].rearrange("p (m q) -> p m q", m=2),
                        AF.Copy,
                    )
                else:
                    nc.vector.tensor_copy(
                        qk_raw[:, 2 * a:2 * a + 2, sl],
                        ps[:].rearrange("p (m q) -> p m q", m=2),
                    )

            def rope(n, m):
                sl = slice(n * 512, (n + 1) * 512)
                ps = ppz.tile([128, 512], F32, tag="ps_z")
                nc.tensor.matmul(
                    ps[:], rot_sb[:], qk_raw[:, m, sl], start=True, stop=True
                )
                t1 = spool.tile([128, 512], BF16, tag="ropet1")
                nc.vector.tensor_tensor(t1[:], ps[:], sin_sb[:, sl],
                                        mybir.AluOpType.mult)
                t2 = spool.tile([128, 512], BF16, tag="ropet2")
                nc.vector.tensor_mul(t2[:], qk_raw[:, m, sl], cos_sb[:, sl])
                nc.vector.tensor_add(qkr[:, m, sl], t1[:], t2[:])

            def proj_phase(mid_emit=None):
                v_ones_memset()
                pending_rots = []
                for n in range(4):
                    for tt in range(4 * n, 4 * n + 2):
                        v_tile(tt)
                    for r in pending_rots:
                        rope(*r)
                    pending_rots = []
                    for tt in range(4 * n + 2, 4 * n + 4):
                        v_tile(tt)
                    qk_pair(n, 0)
                    qk_pair(n, 1)
                    rope(n, 0)
                    rope(n, 1)
                    pending_rots = [(n, 2), (n, 3)]
                    if n == 0 and mid_emit is not None:
                        mid_emit()
                for r in pending_rots:
                    rope(*r)

            # ---- attention, chunk-major ----
            zt_holder = {}

            def attention_chunk(i, mid_emit=None):
                qsl = slice(i * 512, (i + 1) * 512)
                for h in range(HPC):
                    if h == oproj_after_head and mid_emit is not None:
                        mid_emit()
                    rows = slice(64 * (h % 2), 64 * (h % 2) + 64)
                    qm, km = h // 2, 2 + h // 2
                    vc = 128 * h
                    psz = ppz.tile([128, 512], F32, tag="ps_z")
                    started = False
                    # full t-tiles in pairs: one 2-bank psum, one exp
                    for ja in range(0, 4 * i, 2):
                        jb = ja + 1
                        ps2 = ppa.tile([128, 1024], F32, tag="pp")
                        nc.tensor.matmul(
                            ps2[:, 0:512],
                            qkr[rows, km, ja * 128:(ja + 1) * 128],
                            qkr[rows, qm, qsl],
                            start=True, stop=True,
                        )
                        nc.tensor.matmul(
                            ps2[:, 512:1024],
                            qkr[rows, km, jb * 128:(jb + 1) * 128],
                            qkr[rows, qm, qsl],
                            start=True, stop=True,
                        )
                        pt2 = ptpool.tile([128, 1024], BF16, tag="pt")
                        nc.scalar.activation(pt2[:], ps2[:], AF.Exp, scale=0.125)
                        nc.tensor.matmul(
                            psz[:],
                            v_sb[:, ja, vc:vc + 128],
                            pt2[:, 0:512],
                            start=not started, stop=False,
                        )
                        nc.tensor.matmul(
                            psz[:],
                            v_sb[:, jb, vc:vc + 128],
                            pt2[:, 512:1024],
                            start=False, stop=False,
                        )
                        started = True
                    # diagonal: 4 ragged tiles packed into 2 psum pairs.
                    # The causal mask is applied on PE: an identity-lhsT
                    # matmul accumulates -800 into the masked triangle of
                    # each ragged 128-block, so exp underflows it to 0.
                    j0 = 4 * i

                    def diag_tile(ps, c0, j, qlo, last):
                        nc.tensor.matmul(
                            ps[:, c0:c0 + 512 - qlo],
                            qkr[rows, km, j * 128:(j + 1) * 128],
                            qkr[rows, qm, qsl][:, qlo:512],
                            start=True, stop=False,
                        )
                        nc.tensor.matmul(
                            ps[:, c0:c0 + 128],
                            idm_sb[:],
                            tri_sb[:],
                            start=False, stop=last,
                        )

                    psA = ppa.tile([128, 1024], F32, tag="pp")
                    diag_tile(psA, 0, j0, 0, False)
                    diag_tile(psA, 512, j0 + 1, 128, True)
                    ptA = ptpool.tile([128, 1024], BF16, tag="pt")
                    nc.scalar.activation(ptA[:, 0:896], psA[:, 0:896], AF.Exp,
                                         scale=0.125)
                    psB = ppa.tile([128, 1024], F32, tag="pp")
                    diag_tile(psB, 0, j0 + 2, 256, False)
                    diag_tile(psB, 256, j0 + 3, 384, True)
                    ptB = ptpool.tile([128, 1024], BF16, tag="pt")
                    nc.scalar.activation(ptB[:, 0:384], psB[:, 0:384], AF.Exp,
                                         scale=0.125)
                    nc.tensor.matmul(
                        psz[:],
                        v_sb[:, j0, vc:vc + 128],
                        ptA[:, 0:512],
                        start=not started, stop=False,
                    )
                    nc.tensor.matmul(
                        psz[:, 128:512],
                        v_sb[:, j0 + 1, vc:vc + 128],
                        ptA[:, 512:896],
                        start=False, stop=False,
                    )
                    nc.tensor.matmul(
                        psz[:, 256:512],
                        v_sb[:, j0 + 2, vc:vc + 128],
                        ptB[:, 0:256],
                        start=False, stop=False,
                    )
                    nc.tensor.matmul(
                        psz[:, 384:512],
                        v_sb[:, j0 + 3, vc:vc + 128],
                        ptB[:, 256:384],
                        start=False, stop=True,
                    )
                    # psz rows 64-127 hold the softmax denominator
                    zt_t = zt_holder[f"zt{h // 2}"]
                    if use_divide:
                        nc.vector.tensor_tensor(
                            zt_t[rows, qsl],
                            psz[0:64, :],
                            psz[64:128, :],
                            mybir.AluOpType.divide,
                        )
                    else:
                        rsh = spool.tile([64, 512], F32, tag="rsh")
                        nc.vector.reciprocal(rsh[:], psz[64:128, :])
                        nc.vector.tensor_mul(
                            zt_t[rows, qsl],
                            psz[0:64, :],
                            rsh[:],
                        )

            def a2a_send(rnd, _rep, pending_outs):
                # ship zt cols [512*rnd : +512] over all 8 cores; core
                # k = 4b+j keeps cols [512*rnd + 64*k : +64] of EACH batch.
                # Split per head-pair t so the t=0 half (heads 0,1) departs
                # as soon as those heads' norms land.  The PREVIOUS round's
                # output writes are flushed between the t=0 gathers and the
                # t=1 input so they never block a tail-critical DMA on SP.
                col0 = 512 * rnd
                zfs = []
                for t in range(2):
                    zt_t = zt_holder[f"zt{t}"]
                    cc_in = dpool.tile([8, 128, RW], BF16,
                                       tag=f"ccin{rnd}_{t}_{_rep % 2}")
                    cc_out = dpool.tile([8, 128, RW], BF16,
                                        tag=f"ccout{rnd}_{t}_{_rep % 2}")
                    nc.sync.dma_start(
                        cc_in.rearrange("k p q -> p k q"),
                        zt_t[:, col0:col0 + 512].rearrange(
                            "p (k q) -> p k q", k=8),
                    )
                    if for_sim:
                        nc.gpsimd.dma_start(cc_out[:], cc_in[:])
                    else:
                        nc.gpsimd.collective_compute(
                            "AllToAll",
                            mybir.AluOpType.bypass,
                            ins=[cc_in[:].opt()],
                            outs=[cc_out[:].opt()],
                            replica_groups=[list(range(8))],
                        )
                    # zf_t[:, j, b*RW+q] <- cc_out[4b+j][p, q]  (ft = 2j+t)
                    zf = spool.tile([128, 4, 2 * RW], BF16, tag=f"zf{t}_{rnd % 2}")
                    for b in range(2):
                        nc.sync.dma_start(
                            zf[:, :, b * RW:(b + 1) * RW],
                            cc_out[4 * b:4 * b + 4].rearrange("j p q -> p j q"))
                    if t == 0:
                        for dst, src in pending_outs:
                            nc.sync.dma_start(dst, src)
                        pending_outs.clear()
                    zfs.append(zf)
                return zfs

            def oproj_emit(rnd, zfs):
                # contraction over ft = 2j+t, t-major so the t=0 half can
                # start before the t=1 collective completes
                osb = spool.tile([128, 1024], BF16, tag="osb")
                psos = [ppz.tile([128, 512], F32, tag="ps_z", name=f"pso{mc}")
                        for mc in range(2)]
                for t in range(2):
                    for mc in range(2):
                        for j in range(4):
                            nc.tensor.matmul(
                                psos[mc][:],
                                zfs[t][:, j, :],
                                wo_sb[:, 2 * j + t, mc * 512:(mc + 1) * 512],
                                start=(t == 0 and j == 0),
                                stop=(t == 1 and j == 3),
                            )
                outs = []
                for mc in range(2):
                    nc.vector.tensor_copy(osb[:, mc * 512:(mc + 1) * 512],
                                          psos[mc][:])
                    outs.append((
                        out[128 * rnd:128 * (rnd + 1), mc * 512:(mc + 1) * 512],
                        osb[:, mc * 512:(mc + 1) * 512],
                    ))
                return outs

            pending_outs = []
            pending = None
            for _rep in range(reps):
                if _rep == 0:
                    loads()

                def mid_proj():
                    # rep r's last a2a round's o-proj lands inside rep r+1's
                    # projection phase: the collective chain hides under PE
                    # proj work instead of being an exposed tail
                    nonlocal pending
                    if pending is not None:
                        pending_outs.extend(oproj_emit(*pending))
                        pending = None
                proj_phase(mid_emit=mid_proj)
                for t in range(2):
                    zt_holder[f"zt{t}"] = ztpool.tile(
                        [128, S], BF16, name=f"zt{t}", tag=f"zt{t}")
                for i in range(4):
                    if pending is not None:
                        rnd, zf = pending

                        def mid(r=rnd, z=zf):
                            pending_outs.extend(oproj_emit(r, z))
                        pending = None
                        attention_chunk(i, mid_emit=mid)
                    else:
                        attention_chunk(i)
                    zf = a2a_send(i, _rep, pending_outs)
                    pending = (i, zf)
            if pending is not None:
                pending_outs.extend(oproj_emit(*pending))
            for dst, src in pending_outs:
                nc.sync.dma_start(dst, src)

    split_excess_waits(nc)
    return nc


def prepare_in_maps(x, W_qkv, W_o):
    x = np.asarray(x, dtype=np.float32)
    W_qkv = np.asarray(W_qkv, dtype=np.float32)
    W_o = np.asarray(W_o, dtype=np.float32)

    # RoPE tables in the [2-head x dh, seq] transposed layout
    inv_freq = 1.0 / (ROPE_BASE ** (np.arange(0, DH, 2, dtype=np.float32) / DH))
    t = np.arange(S, dtype=np.float32)
    freqs = np.outer(t, inv_freq)                      # [S, 32]
    emb = np.concatenate([freqs, freqs], -1)           # [S, 64]
    cos64 = np.cos(emb).T                              # [64, S]
    sin64 = np.sin(emb).T
    cosp = np.concatenate([cos64, cos64], 0)           # [128, S]
    sinp = np.concatenate([sin64, sin64], 0)

    # rotate-half as a stationary matrix: psum_rot = rotm.T @ qT per 64-block
    r0 = np.zeros((64, 64), dtype=np.float32)
    for dd in range(32):
        r0[dd + 32, dd] = -1.0     # out[d<32] = -q[d+32]
        r0[dd, dd + 32] = 1.0      # out[d>=32] = q[d-32]
    rotm = np.zeros((128, 128), dtype=np.float32)
    rotm[:64, :64] = r0
    rotm[64:, 64:] = r0

    tt_, qq_ = np.meshgrid(np.arange(128), np.arange(128), indexing="ij")
    # additive causal mask: -800 (pre exp-scale 0.125 -> -100) on q < t;
    # exp underflows those entries to exactly 0 in bf16
    trim = (-800.0 * (qq_ < tt_)).astype(np.float32)
    idm = np.eye(128, dtype=np.float32)

    in_maps = []
    for c in range(N_CORES):
        b, g = c // 4, c % 4
        heads = [4 * g + hh for hh in range(HPC)]
        wqk = np.concatenate(
            [W_qkv[:, 64 * h:64 * h + 64] for h in heads]
            + [W_qkv[:, D + 64 * h:D + 64 * h + 64] for h in heads],
            axis=1,
        )
        wv = np.concatenate(
            [W_qkv[:, 2 * D + 64 * h:2 * D + 64 * h + 64] for h in heads], axis=1
        )
        in_maps.append(
            {
                "xt": _npbf(x[b].T),
                "wqk": _npbf(wqk),
                "wv": _npbf(wv),
                "wo": _npbf(W_o),
                "cosp": _npbf(cosp),
                "sinp": _npbf(sinp),
                "rotm": _npbf(rotm),
                "trim": _npbf(trim),
                "idm": _npbf(idm),
            }
        )
    return in_maps


def assemble_output(results):
    full = np.empty((B, S, D), dtype=np.float32)
    for k in range(N_CORES):
        o = np.asarray(results[k]["out"]).astype(np.float32)  # [512, D]
        for rnd in range(4):
            for b in range(B):
                full[b, 512 * rnd + RW * k:512 * rnd + RW * (k + 1)] = \
                    o[128 * rnd + RW * b:128 * rnd + RW * (b + 1)]
    return full


_NC_CACHE = {}


def kernel(x, W_qkv, W_o):
    key = "nc"
    if key not in _NC_CACHE:
        _NC_CACHE[key] = build_nc()
    nc = _NC_CACHE[key]
    in_maps = prepare_in_maps(x, W_qkv, W_o)
    res = run_bass_kernel_spmd(nc, in_maps, core_ids=list(range(N_CORES)))
    return assemble_output(res.results)

